# revision 37
# baseline (speedup 1.0000x reference)
"""Multi-head differential attention Trainium2 kernel (8 NeuronCores).

Sharding: core c -> batch b = c // 4, head group g = c % 4 (4 of 16 heads).
Each core computes its heads' projections, attention, per-head layernorm and
its partial slice of the output projection; the host sums the 4 partials per
batch (standard tensor-parallel unshard) and adds the output bias.

Key optimizations over the v0 baseline:
 - Host-side transpose of q/k/v (straight DMA instead of 24 transpose DMAs).
 - Query compaction: the mask zeroes whole query rows, and every masked row
   yields the SAME per-head output (uniform attention).  The host permutes
   unmasked query rows to the front, appends ONE representative masked row,
   pads to a static width TQP (multiple of 128), and scatters/broadcasts the
   result back.  All query-sided work shrinks by ~TQP/T.
 - mask * 1/sqrt(HS) folded into the host-side xq scaling (projection is
   linear), lambda computed on host (tiny), (1-lambda_init) folded into ln_w/b.
 - bf16 datapath for DVE elementwise work (2x packed mode).
 - Head h+1's q/k projections and the output projection are emitted as PE
   "fillers" inside the (Act-bound) attention kt-loops, so the PE never idles
   waiting on exp; epilogue (softmax-combine + LN) is chunked per q-slice.
 - All matmul PSUM outputs are 2KB-bank-aligned (hardware faults otherwise)
   and never cross a bank boundary.

Math notes (as v0):
 - Masked/padded q columns are zero => scores 0 => exp 1 => uniform softmax,
   identical to the reference's -1e9 masking.
 - Layernorm is invariant to positive per-row scaling, so we feed LN with
       y'' = r2 * y1 - (lam * r1) * y2   (r1/r2 = exp-row-sums; no division).
"""

import math
import sys

sys.path.insert(0, "/opt/trn_rl_repo")

import ml_dtypes
import numpy as np

import concourse.bass as bass
import concourse.bass_isa as bass_isa
import concourse.mybir as mybir
from concourse import bacc
from concourse.bass import ds, ts
from concourse.bass_utils import run_bass_kernel_spmd
from concourse.tile import TileContext

B, T, C, H = 2, 2048, 1024, 16
HS = C // H            # 64
D2 = 2 * HS            # 128
LAYER_IDX = 2
LAMBDA_INIT = 0.8 - 0.6 * float(np.exp(-0.3 * (LAYER_IDX - 1)))
EPS = 1e-9
N_CORES = 8
HPC = H // (N_CORES // B)   # heads per core = 4

FP32 = mybir.dt.float32
BF16 = mybir.dt.bfloat16
AF = mybir.ActivationFunctionType
ALU = mybir.AluOpType

_CACHED = {}


def _q_slices(tqp):
    """Static q-column slices of width <=512 covering [0, tqp)."""
    out, off = [], 0
    while off < tqp:
        w = min(512, tqp - off)
        out.append((off, w))
        off += w
    return out


def _plan_from_mask(mask_row):
    """-> (perm[tqp_need], n1).  perm = unmasked rows, then one representative
    masked row (if any), the caller pads to the static TQP."""
    mask_row = np.asarray(mask_row)
    unm = np.flatnonzero(mask_row != 0)
    msk = np.flatnonzero(mask_row == 0)
    perm = np.concatenate([unm, msk[:1]]).astype(np.int64)
    return perm, len(unm)


def _choose_tqp(mask):
    # multiple of 8 (16B-aligned bf16 rows); cap at T
    need = max(len(_plan_from_mask(mask[b])[0]) for b in range(B))
    return int(min(max((need + 7) // 8 * 8, 128), T))


def _row_tiles(tqp):
    """128-row output tiles, last one possibly partial."""
    out, off = [], 0
    while off < tqp:
        r = min(128, tqp - off)
        out.append((off, r))
        off += r
    return out


def build_nc(repeat=1, tqp=None):
    if tqp is None:
        tqp = _CACHED.get("tqp", T)
    nc = bacc.Bacc("TRN2", target_bir_lowering=False, debug=False,
                   enable_asserts=False)

    # host pre-transposed (and for q: mask*scale-folded, compacted) inputs
    xqt_d = nc.dram_tensor("xqt", [C, tqp], BF16, kind="ExternalInput").ap()
    xkt_d = nc.dram_tensor("xkt", [C, T], BF16, kind="ExternalInput").ap()
    xvt_d = nc.dram_tensor("xvt", [C, T], BF16, kind="ExternalInput").ap()
    # weights, host packed to SBUF layout (partition dim first)
    wq_d = nc.dram_tensor("wq", [128, HPC * 8 * 128], BF16, kind="ExternalInput").ap()
    wk_d = nc.dram_tensor("wk", [128, HPC * 8 * 128], BF16, kind="ExternalInput").ap()
    wv_d = nc.dram_tensor("wv", [128, 8 * 512], BF16, kind="ExternalInput").ap()
    wc_d = nc.dram_tensor("wc", [128, HPC * 1024], BF16, kind="ExternalInput").ap()
    lnw_d = nc.dram_tensor("lnw", [128, 1], FP32, kind="ExternalInput").ap()
    lnb_d = nc.dram_tensor("lnb", [128, 1], FP32, kind="ExternalInput").ap()
    lam_d = nc.dram_tensor("lam", [1, HPC], FP32, kind="ExternalInput").ap()
    out_d = nc.dram_tensor("out", [tqp, C], FP32, kind="ExternalOutput").ap()

    QSL = _q_slices(tqp)
    NKT = T // 128     # 16 k tiles
    # large tqp (rare fallback; only hit if the mask is far less sparse than
    # the expected ~50%): shrink buffering to fit SBUF
    sb = 1 if tqp > 1400 else 2
    eb = 2 if tqp > 1400 else 3

    with TileContext(nc) as tc:
      for _rep in range(repeat):
        with (
            tc.tile_pool(name="singles", bufs=1) as singles,
            tc.tile_pool(name="proj", bufs=1) as proj_pool,
            tc.tile_pool(name="xpool", bufs=1) as x_pool,
            tc.tile_pool(name="ppsum", bufs=2, space="PSUM") as ppsum,
        ):
            # ---------- constants / tiny prep ----------
            lnw_sb = singles.tile([128, 1], FP32, tag="lnw")
            lnb_sb = singles.tile([128, 1], FP32, tag="lnb")
            nc.sync.dma_start(out=lnw_sb, in_=lnw_d)
            nc.sync.dma_start(out=lnb_sb, in_=lnb_d)
            lam_row = singles.tile([1, HPC], FP32, tag="lam_row")
            nc.sync.dma_start(out=lam_row, in_=lam_d)
            lam_col = singles.tile([128, HPC], FP32, tag="lam_col")
            nc.gpsimd.partition_broadcast(lam_col, lam_row, 128)
            eps_col = singles.tile([128, 1], FP32, tag="eps_col")
            nc.vector.memset(eps_col, EPS)

            qmapT = [proj_pool.tile([128, tqp], BF16, tag=f"qm{h}", name=f"qm{h}")
                     for h in range(HPC)]
            kmapT = [proj_pool.tile([128, T], BF16, tag=f"km{h}", name=f"km{h}")
                     for h in range(HPC)]
            vv = [proj_pool.tile([128, 4 * D2], BF16, tag=f"vv{i}", name=f"vv{i}")
                  for i in range(NKT)]
            ynormT = [proj_pool.tile([128, tqp], BF16, tag=f"yn{h}", name=f"yn{h}")
                      for h in range(HPC)]

            wq_sb = x_pool.tile([128, HPC * 8 * 128], BF16, tag="wq")
            wk_sb = x_pool.tile([128, HPC * 8 * 128], BF16, tag="wk")

            def w_qk(w_sb, h, ct):   # [128, 128] lhsT (C-tile ct, head h)
            	return w_sb[:, ds((h * 8 + ct) * 128, 128)]

            def load_xt(pool, x_d, nm, width):
                # alternate tiles across both HWDGE queues (SP + Activation)
                tiles = []
                for i in range(8):
                    xt = pool.tile([128, width], BF16, tag=f"{nm}{i}",
                                   name=f"{nm}{i}")
                    eng = nc.sync if i % 2 == 0 else nc.scalar
                    eng.dma_start(out=xt, in_=x_d[ds(i * 128, 128), :])
                    tiles.append(xt)
                return tiles

            # DMAs in first-use order, split across the two HWDGE queues;
            # wq in halves so head 0's first chain starts after half a load
            half = HPC * 4 * 128
            nc.sync.dma_start(out=wq_sb[:, 0:half], in_=wq_d[:, 0:half])
            nc.scalar.dma_start(out=wk_sb, in_=wk_d)
            xqT = load_xt(x_pool, xqt_d, "xq", tqp)
            nc.sync.dma_start(out=wq_sb[:, half:2 * half],
                              in_=wq_d[:, half:2 * half])
            xkT = load_xt(x_pool, xkt_d, "xk", T)

            # projection chunk emitters (also used as PE fillers during
            # attention: head h+1's q/k projections run in head h's Act-bound
            # stalls)
            def _proj_chunk(w_sb, xT, h, off, w, dst, cp, split):
                # optionally split the 8-matmul accumulation chain into two
                # half-chain closures (finer-grained PE fillers); foreign
                # matmuls may interleave between halves (separate psum banks)
                cell = {}

                def emit_a():
                    cell["ps"] = ppsum.tile([128, 512], FP32, tag="ppsum",
                                            name="ps")
                    ps = cell["ps"][:, 0:w]
                    for ct in range(4):
                        nc.tensor.matmul(ps, w_qk(w_sb, h, ct),
                                         xT[ct][:, ds(off, w)],
                                         start=(ct == 0), stop=False,
                                         skip_group_check=True)

                def emit_b():
                    ps = cell["ps"][:, 0:w]
                    for ct in range(4, 8):
                        nc.tensor.matmul(ps, w_qk(w_sb, h, ct),
                                         xT[ct][:, ds(off, w)],
                                         start=False, stop=(ct == 7),
                                         skip_group_check=True)
                    cp(dst, ps)

                if split:
                    return [emit_a, emit_b]
                return [lambda: (emit_a(), emit_b())]

            def qk_chunks(h, cp, split=False):
                out = []
                for off, w in QSL:
                    out += _proj_chunk(wq_sb, xqT, h, off, w,
                                       qmapT[h][:, ds(off, w)], cp, split)
                for ks in range(4):
                    out += _proj_chunk(wk_sb, xkT, h, ks * 512, 512,
                                       kmapT[h][:, ds(ks * 512, 512)], cp,
                                       split)
                return out

            # head 0's projections run at startup while Act is idle
            for f in qk_chunks(0, nc.scalar.copy):
                f()

            # ---------- v projection (own scope; frees x_v SBUF after) ----
            with tc.tile_pool(name="vload", bufs=1) as v_pool:
                wv_sb = v_pool.tile([128, 8 * 512], BF16, tag="wv")
                nc.scalar.dma_start(out=wv_sb, in_=wv_d)
                xvT = load_xt(v_pool, xvt_d, "xv", T)
                for kt in range(NKT):
                    ps = ppsum.tile([128, 512], FP32, tag="ppsum")
                    for ct in range(8):
                        nc.tensor.matmul(ps, xvT[ct][:, ds(kt * 128, 128)],
                                         wv_sb[:, ds(ct * 512, 512)],
                                         start=(ct == 0), stop=(ct == 7))
                    nc.scalar.copy(vv[kt], ps)

            # wc loads late (only needed by the output projection)
            wc_sb = singles.tile([128, HPC * 1024], BF16, tag="wc")
            nc.sync.dma_start(out=wc_sb, in_=wc_d)

            # output-projection row-tile emitters: interleaved into head 3's
            # attention once all heads' ynorm for the rows is final
            ob_pool = proj_pool
            def o_chunk(off, r):
                def emit():
                    qsl = ds(off, r)
                    ob = ob_pool.tile([r, C], FP32, tag="ob", name="ob",
                                      bufs=2)
                    for cs in range(2):
                        ps = ppsum.tile([r, 512], FP32, tag="ppsum",
                                        name="ps")
                        for h in range(HPC):
                            nc.tensor.matmul(ps, ynormT[h][:, qsl],
                                             wc_sb[:, ds(h * 1024 + cs * 512, 512)],
                                             start=(h == 0), stop=(h == HPC - 1))
                        nc.vector.tensor_copy(ob[:, ds(cs * 512, 512)], ps)
                    nc.sync.dma_start(out=out_d[qsl, :], in_=ob)
                return emit

            # ---------- attention (chunked epilogue per (h, q-slice)) ------
            with (
                tc.tile_pool(name="escr", bufs=eb) as e_pool,
                tc.tile_pool(name="scr", bufs=sb) as scr_pool,
                tc.tile_pool(name="spsum", bufs=2, space="PSUM") as spsum,
                tc.tile_pool(name="ypsum", bufs=2, space="PSUM") as ypsum,
            ):
                OT = _row_tiles(tqp)
                o_next = 0          # next out-proj row tile to schedule
                for h in range(HPC):
                    vslice = ds(h * D2, D2)
                    last_h = h + 1 == HPC
                    fillers = ([] if last_h else
                               list(reversed(qk_chunks(h + 1,
                                                       nc.vector.tensor_copy,
                                                       split=True))))
                    for qi, (off, w) in enumerate(QSL):
                        if last_h:
                            # rows fully covered by this head's UPPER slices
                            avail = sum(ww for _, ww in QSL[:qi])
                            while (o_next < len(OT)
                                   and OT[o_next][0] + OT[o_next][1] <= avail):
                                fillers.insert(0, o_chunk(*OT[o_next]))
                                o_next += 1
                        qsl = ds(off, w)
                        y1f = ypsum.tile([128, 512], FP32, tag="y")
                        y2f = ypsum.tile([128, 512], FP32, tag="y")
                        y1, y2 = y1f[:, 0:w], y2f[:, 0:w]
                        ra0 = scr_pool.tile([128, 2 * w], BF16, tag="ra0")
                        ra1 = scr_pool.tile([128, 2 * w], BF16, tag="ra1")
                        for kt in range(NKT):
                            ksl = ds(kt * 128, 128)
                            # scores psum: map1 at col 0, map2 at col 512
                            # (bank boundary: matmul PSUM writes must be
                            # bank-aligned or hardware faults)
                            s = spsum.tile([128, 1024], FP32, tag="s")
                            nc.tensor.matmul(s[:, 0:w],
                                             kmapT[h][0:64, ksl],
                                             qmapT[h][0:64, qsl],
                                             start=True, stop=True,
                                             tile_position=(0, 0))
                            nc.tensor.matmul(s[:, 512:512 + w],
                                             kmapT[h][64:128, ksl],
                                             qmapT[h][64:128, qsl],
                                             start=True, stop=True,
                                             tile_position=(64, 0))
                            if fillers and kt % 3 == 2:
                                fillers.pop()()
                            e = e_pool.tile([128, 2 * w], BF16, tag="e")
                            sv = s.rearrange("p (two q) -> p two q", two=2,
                                             q=512)[:, :, 0:w]
                            ev = e.rearrange("p (two q) -> p two q", two=2,
                                             q=w)
                            nc.scalar.activation(ev, sv, AF.Exp)
                            nc.tensor.matmul(y1, vv[kt][:, vslice], e[:, 0:w],
                                             start=(kt == 0), stop=(kt == NKT - 1))
                            nc.tensor.matmul(y2, vv[kt][:, vslice],
                                             e[:, w:2 * w],
                                             start=(kt == 0), stop=(kt == NKT - 1))
                            # exp-row-sum accumulation: two DVE chains
                            tgt = ra0 if kt % 2 == 0 else ra1
                            if kt < 2:
                                nc.vector.tensor_copy(tgt, e)
                            else:
                                nc.vector.tensor_add(tgt, tgt, e)

                        # ---- chunk epilogue: [128, w] wide ----
                        # y copies first: frees the ypsum banks for the next
                        # q-slice's accumulation as early as possible
                        y1c = scr_pool.tile([128, w], BF16, tag="y1c")
                        y2c = scr_pool.tile([128, w], BF16, tag="y2c")
                        nc.vector.tensor_copy(y1c, y1)
                        nc.vector.tensor_copy(y2c, y2)
                        rc = scr_pool.tile([128, 2 * w], BF16, tag="rc")
                        nc.vector.tensor_add(rc, ra0, ra1)
                        rallc = scr_pool.tile([128, 2 * w], FP32, tag="rall")
                        nc.gpsimd.partition_all_reduce(rallc, rc, 128,
                                                       bass_isa.ReduceOp.add)
                        rbc = scr_pool.tile([128, 2 * w], BF16, tag="rbc")
                        nc.vector.tensor_copy(rbc, rallc)
                        # y'' = r2*y1 - (lam*r1)*y2  (LN is scale-invariant)
                        c2 = scr_pool.tile([128, w], BF16, tag="c2")
                        nc.vector.tensor_scalar(c2, rbc[:, 0:w],
                                                lam_col[:, ds(h, 1)],
                                                None, op0=ALU.mult)
                        slnc = scr_pool.tile([128, 2 * w], BF16, tag="sln")
                        ylnc = slnc[:, 0:w]
                        ysqc = slnc[:, w:2 * w]
                        nc.vector.tensor_mul(y1c, y1c, rbc[:, w:2 * w])
                        nc.vector.tensor_mul(c2, y2c, c2)
                        nc.vector.tensor_sub(ylnc, y1c, c2)
                        nc.vector.tensor_mul(ysqc, ylnc, ylnc)
                        sredc = scr_pool.tile([128, 2 * w], FP32, tag="sred")
                        nc.gpsimd.partition_all_reduce(sredc, slnc, 128,
                                                       bass_isa.ReduceOp.add)
                        mean = scr_pool.tile([128, w], FP32, tag="mean")
                        meanb = scr_pool.tile([128, w], BF16, tag="meanb")
                        var = scr_pool.tile([128, w], FP32, tag="var")
                        nc.vector.tensor_scalar(mean, sredc[:, 0:w], 1.0 / D2,
                                                None, op0=ALU.mult)
                        nc.vector.tensor_scalar(var, sredc[:, w:2 * w],
                                                1.0 / D2, None, op0=ALU.mult)
                        nc.vector.tensor_copy(meanb, mean)
                        msq = scr_pool.tile([128, w], FP32, tag="msq")
                        nc.vector.tensor_mul(msq, mean, mean)
                        nc.vector.tensor_sub(var, var, msq)
                        # rstd = exp(-0.5 * ln(var + eps)) (bf16 out)
                        rstdb = scr_pool.tile([128, w], BF16, tag="rstdb")
                        nc.scalar.activation(var, var, AF.Ln, bias=eps_col)
                        nc.scalar.activation(rstdb, var, AF.Exp, scale=-0.5)
                        nc.vector.tensor_sub(ylnc, ylnc, meanb)
                        nc.vector.tensor_mul(ylnc, ylnc, rstdb)
                        nc.vector.tensor_scalar(ynormT[h][:, qsl], ylnc,
                                                lnw_sb, lnb_sb,
                                                op0=ALU.mult, op1=ALU.add)
                    # drain any unemitted fillers for this head
                    while fillers:
                        fillers.pop()()
                # remaining out-proj row tiles (last q-slice region)
                while o_next < len(OT):
                    o_chunk(*OT[o_next])()
                    o_next += 1

    # Force every activation (Exp + Ln) onto the combined
    # natural_log_exp_and_others table set so the epilogue's Ln/Exp pair
    # doesn't thrash ACT_TABLE_LOADs against the attention Exps (~2.7us per
    # switch).  Emptying the other sets keeps the set-id numbering intact.
    _orig_tables = bacc.get_activation_tables

    def _only_combined(arch):
        out = {}
        for name, funcs in _orig_tables(arch).items():
            out[name] = funcs if name == "natural_log_exp_and_others" else set()
        return out

    bacc.get_activation_tables = _only_combined
    try:
        nc.compile()
    finally:
        bacc.get_activation_tables = _orig_tables
    return nc


def _prep_core_inputs(inputs, core, tqp=None):
    if tqp is None:
        tqp = _choose_tqp(inputs["mask"])
    b = core // (N_CORES // B)
    g = core % (N_CORES // B)
    h2 = slice(g * HPC * D2, (g + 1) * HPC * D2)          # 128/head cols
    bf = ml_dtypes.bfloat16

    perm, n1 = _plan_from_mask(inputs["mask"][b])
    if len(perm) < tqp:   # pad with copies of the last (masked rep) row
        pad = np.full(tqp - len(perm), perm[-1] if len(perm) else 0,
                      np.int64)
        perm = np.concatenate([perm, pad])
    # compacted, mask*scale-folded q^T;  plain k^T, v^T
    colscale = (inputs["mask"][b][perm] != 0).astype(np.float32) / math.sqrt(HS)
    xqt = (inputs["q"][b].T[:, perm] * colscale[None, :]).astype(bf)
    xkt = np.ascontiguousarray(inputs["k"][b].T).astype(bf)
    xvt = np.ascontiguousarray(inputs["v"][b].T).astype(bf)

    def pack_qk(w1, w2):
        # -> [128, HPC*8*128]: per head the 8 C-tiles of [Wq1_h | Wq2_h]
        cols = []
        for h in range(HPC):
            hh = slice((g * HPC + h) * HS, (g * HPC + h + 1) * HS)
            w = np.concatenate([w1[:, hh], w2[:, hh]], axis=1)   # [1024, 128]
            cols.append(w.reshape(8, 128, 128))
        arr = np.stack(cols, 0)                    # [HPC, 8, 128, 128]
        return np.ascontiguousarray(
            arr.transpose(2, 0, 1, 3).reshape(128, -1)).astype(bf)

    wv = inputs["Wv"][:, h2].reshape(8, 128, HPC * D2)
    wv = np.ascontiguousarray(wv.transpose(1, 0, 2).reshape(128, -1)).astype(bf)
    wc = inputs["Wc"][h2, :].reshape(HPC, 128, C)
    wc = np.ascontiguousarray(wc.transpose(1, 0, 2).reshape(128, -1)).astype(bf)

    heads = slice(g * HPC, (g + 1) * HPC)
    lam = (np.exp((inputs["lq1"][heads] * inputs["lk1"][heads]).sum(-1))
           - np.exp((inputs["lq2"][heads] * inputs["lk2"][heads]).sum(-1))
           + LAMBDA_INIT).astype(np.float32)

    sc = np.float32(1.0 - LAMBDA_INIT)
    return {
        "xqt": xqt,
        "xkt": xkt,
        "xvt": xvt,
        "wq": pack_qk(inputs["Wq1"], inputs["Wq2"]),
        "wk": pack_qk(inputs["Wk1"], inputs["Wk2"]),
        "wv": wv,
        "wc": wc,
        "lnw": (inputs["ln_w"] * sc).astype(np.float32).reshape(128, 1),
        "lnb": (inputs["ln_b"] * sc).astype(np.float32).reshape(128, 1),
        "lam": lam.reshape(1, HPC),
    }


def kernel(q, k, v, mask, Wq1, bq1, Wq2, bq2, Wk1, bk1, Wk2, bk2,
           Wv, bv, Wc, bc, ln_w, ln_b, lq1, lk1, lq2, lk2, **run_kw):
    inputs = dict(q=np.asarray(q), k=np.asarray(k), v=np.asarray(v),
                  mask=np.asarray(mask), Wq1=np.asarray(Wq1),
                  Wq2=np.asarray(Wq2), Wk1=np.asarray(Wk1), Wk2=np.asarray(Wk2),
                  Wv=np.asarray(Wv), Wc=np.asarray(Wc),
                  ln_w=np.asarray(ln_w), ln_b=np.asarray(ln_b),
                  lq1=np.asarray(lq1), lk1=np.asarray(lk1),
                  lq2=np.asarray(lq2), lk2=np.asarray(lk2))
    tqp = _choose_tqp(inputs["mask"])
    _CACHED["tqp"] = tqp
    key = ("nc", tqp)
    if key not in _CACHED:
        _CACHED[key] = build_nc(tqp=tqp)
    nc = _CACHED[key]
    in_maps = [_prep_core_inputs(inputs, c, tqp) for c in range(N_CORES)]
    res = run_bass_kernel_spmd(nc, in_maps, list(range(N_CORES)), **run_kw)
    _CACHED["last_results"] = res
    gpb = N_CORES // B
    out = np.zeros((B, T, C), np.float32)
    for b in range(B):
        acc = np.zeros((tqp, C), np.float32)
        for g in range(gpb):
            acc += res.results[b * gpb + g]["out"]
        perm, n1 = _plan_from_mask(inputs["mask"][b])
        unm = perm[:n1]
        out[b, unm] = acc[:n1]
        msk = np.flatnonzero(inputs["mask"][b] == 0)
        if len(msk):
            out[b, msk] = acc[n1]
        out[b] += np.asarray(bc, np.float32)[None, :]
    return out
